# revision 26
# baseline (speedup 1.0000x reference)
"""Trainium2 Bass kernel for nn_CausalNet (block-diagonal GNN + BN + MLP head).

Strategy: data-parallel over batch (8 samples/core on 8 cores).
 - Feature-major layouts throughout so every BN/broadcast is per-partition.
 - A = outer(rinv) * ((mA + I) * Gram) exactly (Gram diagonal is ||x||^2, so
   the self-loop identity folds into the mask constant); An = outer(rinv*dinv)
   * (same masked Gram). Partition-axis reductions (row norms, degrees) via
   ones-vector matmuls; all heavy matmuls in bf16 (PSUM accumulates fp32).
 - X@W precomputed for all blocks up front (dense tensor-engine schedule, the
   PE pstate-ramps to full clock only when continuously busy).
 - BatchNorm stats: per-partition sums accumulated during PSUM->SBUF copies
   (scalar Copy accum_out), sumsq via one tensor_tensor_reduce; 2KB AllReduce.
 - Readout [64,131072]@[131072,128]: h2 (bf16) AllToAll'd per-feature-half so
   each core contracts its natural 16384-row slice of Wm1 (resident bf16 in
   SBUF); import is contiguous feature-major DMA; the patch-stride is folded
   into the matmul rhs AP. 32KB AllReduce combines partial z1^T; head is
   transpose-free and replicated.
"""
import sys
import numpy as np

sys.path.insert(0, "/opt/trn_rl_repo")

B, N, P, D = 64, 4, 128, 256
H = 256
TOTP = N * P          # 512
NCORES = 8
BLOC = B // NCORES    # 8 samples per core
T = BLOC * TOTP       # 4096 tokens per core
NB = BLOC * N         # 32 (sample, subgraph) blocks per core
FEAT = TOTP * H       # 131072
FSL = FEAT // NCORES  # 16384 Wm1 rows per core
TSL = TOTP // NCORES  # 64 patches per core slice
JT = H // 128         # 2 feature partition-tiles
EPS_BN = 1e-5
CNT1 = float(B * TOTP)   # BN denominator for GCN layers
CNT2 = float(B)          # BN denominator for head
SB = NB // 4             # 8 groups of 4 blocks (512 columns each)


def build_bass(has_bias=False, repeat=1, no_cc=False):
    import concourse.bass as bass
    import concourse.bacc as bacc
    import concourse.mybir as mybir
    import concourse.tile as tile

    f32 = mybir.dt.float32
    bf16 = mybir.dt.bfloat16
    Act = mybir.ActivationFunctionType
    Alu = mybir.AluOpType
    AX = mybir.AxisListType

    nc = bacc.Bacc("TRN2", target_bir_lowering=False, debug=False,
                   num_devices=NCORES)

    def inp(name, shape, dt=f32):
        return nc.dram_tensor(name, shape, dt, kind="ExternalInput")

    xT_d = inp("xT", [D, T], bf16)      # d-major activations for this core
    W1_d = inp("W1", [D, H], bf16)
    b1_d = inp("b1r", [1, H], bf16)
    g1_d = inp("g1p", [128, JT])        # column jh = features [jh*128,(jh+1)*128)
    be1_d = inp("be1p", [128, JT])
    W2_d = inp("W2", [H, H], bf16)
    b2_d = inp("b2r", [1, H], bf16)
    g2_d = inp("g2p", [128, JT])
    be2_d = inp("be2p", [128, JT])
    mAT_d = inp("mAT", [P, 4 * P], bf16)  # (0.5*mask*(1-I)).T tiled x4
    mBT_d = inp("mBT", [P, 4 * P], bf16)  # (0.5*mask*(1-I) + I).T tiled x4
    Wm1_d = inp("Wm1s", [FSL, 128], bf16)  # natural-order Wm1 row slice
    gm1_d = inp("gm1", [128, 1])
    bem1_d = inp("bem1", [128, 1])
    Wm2_d = inp("Wm2", [128, 64])
    gm2_d = inp("gm2", [64, 1])
    bem2_d = inp("bem2", [64, 1])
    Wm3_d = inp("Wm3", [64, 2])
    bm3_d = inp("bm3", [2, 1])
    oner_d = inp("ones_row", [1, 128], bf16)

    out_ext = nc.dram_tensor("out", [2, B], f32, kind="ExternalOutput")

    with tile.TileContext(nc) as tc:
        with (
            tc.tile_pool(name="persist", bufs=1) as pp,
            tc.tile_pool(name="work", bufs=3) as wp,
            tc.tile_pool(name="small", bufs=2) as sp,
            tc.tile_pool(name="ps", bufs=2, space="PSUM") as ps,
            tc.tile_pool(name="dram", bufs=1, space="DRAM") as dp,
        ):
            # ---------------- persistent SBUF ----------------
            def load(name, dram, shape, sl=None, dt=f32):
                t = pp.tile(shape, dt, tag=name, name=name)
                nc.gpsimd.dma_start(out=t[:], in_=dram[:] if sl is None else sl)
                return t

            xTs = [load(f"xT{k}", xT_d, [128, T], xT_d[k * 128:(k + 1) * 128, :],
                        dt=bf16) for k in range(2)]
            W1s = [load(f"W1{k}", W1_d, [128, H], W1_d[k * 128:(k + 1) * 128, :],
                        dt=bf16) for k in range(2)]
            W2s = [load(f"W2{k}", W2_d, [128, H], W2_d[k * 128:(k + 1) * 128, :],
                        dt=bf16) for k in range(2)]
            b1s = load("b1", b1_d, [1, H], dt=bf16)
            b2s = load("b2", b2_d, [1, H], dt=bf16)
            mATs = load("mAT", mAT_d, [P, 4 * P], dt=bf16)
            mBTs = load("mBT", mBT_d, [P, 4 * P], dt=bf16)
            oner = load("oner", oner_d, [1, 128], dt=bf16)
            g1s = load("g1", g1_d, [128, JT])
            be1s = load("be1", be1_d, [128, JT])
            g2s = load("g2", g2_d, [128, JT])
            be2s = load("be2", be2_d, [128, JT])
            gm1s = load("gm1", gm1_d, [128, 1])
            bem1s = load("bem1", bem1_d, [128, 1])
            gm2s = load("gm2", gm2_d, [64, 1])
            bem2s = load("bem2", bem2_d, [64, 1])
            Wm2s = load("Wm2", Wm2_d, [128, 64])
            Wm3s = load("Wm3", Wm3_d, [64, 2])
            bm3s = load("bm3", bm3_d, [2, 1])
            # Wm1 slice resident bf16: [f, (t_loc jh o)] from rows (t*256+jh*128+f)
            wm1s = pp.tile([128, TSL * JT * 128], bf16, tag="wm1", name="wm1s")
            wm1_src = Wm1_d.rearrange("(tj f) o -> f tj o", f=128)
            wm1_dst = wm1s.rearrange("f (tj o) -> f tj o", o=128)
            for q in range(4):
                nc.gpsimd.dma_start(out=wm1_dst[:, q * 32:(q + 1) * 32, :],
                                    in_=wm1_src[:, q * 32:(q + 1) * 32, :])

            onecb = pp.tile([128, 1], bf16, tag="onecb")
            nc.vector.memset(onecb[:], 1.0)
            epsb = pp.tile([128, 1], f32, tag="epsb")
            nc.vector.memset(epsb[:], EPS_BN)
            AnT = pp.tile([128, NB * P], bf16, tag="AnT")     # scaled A^T blocks
            h1T = [pp.tile([128, T], bf16, tag=f"h1T{k}", name=f"h1T{k}")
                   for k in range(JT)]
            h2T = [pp.tile([128, T], bf16, tag=f"h2T{k}", name=f"h2T{k}")
                   for k in range(JT)]

            rg = [list(range(NCORES))]

            def cc(kind, op, cin, cout):
                if no_cc:
                    nc.sync.dma_start(out=cout[:], in_=cin[:])
                else:
                    nc.gpsimd.collective_compute(
                        kind, op, replica_groups=rg,
                        ins=[cin.opt()], outs=[cout.opt()])

            # X@W for all 32 blocks -> xw_all[t, (blk, h)] bf16 (dense PE phase)
            def xw_layer(srcs, Ws, bs, tag):
                xw_all = pp.tile([128, NB * H], bf16, tag="xwall", name=tag)
                for blk in range(NB):
                    cb = blk * P
                    xw_ps = ps.tile([128, H], f32, tag="xw")
                    nkt = len(srcs)
                    for kt in range(nkt):
                        nc.tensor.matmul(
                            xw_ps[:], srcs[kt][:, cb:cb + P], Ws[kt][:],
                            start=(kt == 0), stop=(kt == nkt - 1 and not has_bias),
                        )
                    if has_bias:
                        nc.tensor.matmul(xw_ps[:], oner[:], bs[:],
                                         start=False, stop=True)
                    nc.vector.tensor_copy(xw_all[:, blk * H:(blk + 1) * H], xw_ps[:])
                return xw_all

            # copy hh psum into hT + incremental BN stat partials for the slice
            def hh_copy(hT, jh, c0, hh4, stp, sb):
                nc.vector.tensor_copy(hT[jh][:, c0:c0 + 4 * P], hh4[:])
                nc.vector.reduce_sum(stp[:, sb * 4 + jh:sb * 4 + jh + 1],
                                     hT[jh][:, c0:c0 + 4 * P], AX.X)
                sqo = wp.tile([128, 512], bf16, tag="sqc")
                nc.scalar.activation(sqo[:], hT[jh][:, c0:c0 + 4 * P], Act.Square,
                                     accum_out=stp[:, sb * 4 + 2 + jh:sb * 4 + 3 + jh])

            def bn_stats_ar(stp, stin, stout):
                st = sp.tile([128, 4], f32, tag="st")
                stv = stp.rearrange("p (s k) -> p s k", k=4)
                for k in range(4):
                    nc.vector.reduce_sum(st[:, k:k + 1], stv[:, :, k], AX.X)
                nc.gpsimd.dma_start(out=stin[:], in_=st[:])
                nc.gpsimd.collective_compute(
                    "AllReduce", Alu.add, replica_groups=rg,
                    ins=[stin.opt()], outs=[stout.opt()],
                )
                stg = sp.tile([128, 4], f32, tag="stg")
                nc.gpsimd.dma_start(out=stg[:], in_=stout[:])
                return stg

            def bn_params(stg, gs, bes, jh):
                mean = sp.tile([128, 1], f32, tag="mean")
                nc.vector.tensor_scalar_mul(mean[:], stg[:, jh:jh + 1], 1.0 / CNT1)
                msq = sp.tile([128, 1], f32, tag="msq")
                nc.vector.tensor_mul(msq[:], mean[:], mean[:])
                var = sp.tile([128, 1], f32, tag="var")
                nc.vector.tensor_scalar_mul(var[:], stg[:, 2 + jh:3 + jh], 1.0 / CNT1)
                nc.vector.tensor_sub(var[:], var[:], msq[:])
                sd = sp.tile([128, 1], f32, tag="sd")
                nc.scalar.activation(sd[:], var[:], Act.Sqrt,
                                     bias=epsb[:var.shape[0], :])
                rsd = sp.tile([128, 1], f32, tag="rsd")
                nc.vector.reciprocal(rsd[:], sd[:])
                a = sp.tile([128, 1], f32, tag=f"a{jh}", bufs=1)
                nc.vector.tensor_mul(a[:], gs[:, jh:jh + 1], rsd[:])
                c = sp.tile([128, 1], f32, tag=f"c{jh}", bufs=1)
                nc.vector.tensor_mul(c[:], mean[:], a[:])
                nc.vector.tensor_sub(c[:], bes[:, jh:jh + 1], c[:])
                return a, c

            def bn_gcn(hT, stp, stin, stout, gs, bes):
                stg = bn_stats_ar(stp, stin, stout)
                for jh in range(JT):
                    a, c = bn_params(stg, gs, bes, jh)
                    for ch in range(SB):
                        nc.scalar.activation(
                            hT[jh][:, ch * 512:(ch + 1) * 512],
                            hT[jh][:, ch * 512:(ch + 1) * 512],
                            Act.Relu, bias=c[:], scale=a[:])

            for _rep in range(repeat):
                st1_in = dp.tile([128, 4], f32, tag="st1i", name="st1_in")
                st1_out = dp.tile([128, 4], f32, tag="st1o", addr_space="Shared",
                                  name="st1_out")
                st2_in = dp.tile([128, 4], f32, tag="st2i", name="st2_in")
                st2_out = dp.tile([128, 4], f32, tag="st2o", addr_space="Shared",
                                  name="st2_out")
                a2a_in = [dp.tile([NCORES, BLOC, 128, TSL], bf16, tag=f"a2ai{j}",
                                  name=f"a2a_in{j}") for j in range(JT)]
                a2a_out = [dp.tile([NCORES, BLOC, 128, TSL], bf16, tag=f"a2ao{j}",
                                   name=f"a2a_out{j}") for j in range(JT)]
                z1_in = dp.tile([128, B], f32, tag="z1i", name="z1_in")
                z1_out = dp.tile([128, B], f32, tag="z1o", addr_space="Shared",
                                 name="z1_out")

                # ======== layer 1 ========
                xw1 = xw_layer(xTs, W1s, b1s, "xw1")
                st1p = sp.tile([128, 32], f32, tag="stp", bufs=1)

                for sb in range(SB):
                    c0 = sb * 4 * P
                    # row norms for this 512-column chunk via ones-matmul reduce
                    nrm_ps = ps.tile([1, 512], f32, tag="hh")
                    for kt in range(2):
                        sq = wp.tile([128, 512], bf16, tag="sqc")
                        nc.scalar.activation(sq[:], xTs[kt][:, c0:c0 + 512],
                                             Act.Square)
                        nc.tensor.matmul(nrm_ps[:], onecb[:], sq[:],
                                         start=(kt == 0), stop=(kt == 1))
                    nr = sp.tile([1, 512], f32, tag="nr", bufs=2)
                    nc.scalar.activation(nr[:], nrm_ps[:], Act.Sqrt)
                    rinv = sp.tile([1, 512], f32, tag="rinv", bufs=2)
                    nc.vector.reciprocal_approx_fast(rinv[:], nr[:])
                    rbf = sp.tile([1, 512], bf16, tag="rbf", bufs=2)
                    nc.vector.tensor_copy(rbf[:], rinv[:])

                    # gram for 4 blocks
                    G4 = ps.tile([P, 4 * P], f32, tag="G")
                    for b in range(4):
                        cb = c0 + b * P
                        for kt in range(2):
                            nc.tensor.matmul(
                                G4[:, b * P:(b + 1) * P],
                                xTs[kt][:, cb:cb + P], xTs[kt][:, cb:cb + P],
                                start=(kt == 0), stop=(kt == 1),
                            )
                    t1 = wp.tile([P, 4 * P], bf16, tag="t1")
                    nc.vector.tensor_mul(t1[:], G4[:], mATs[:])

                    # A^T = outer(rinv) * t1 + mB^T (mB carries mA + I)
                    R4 = ps.tile([P, 4 * P], f32, tag="adj")
                    for b in range(4):
                        nc.tensor.matmul(R4[:, b * P:(b + 1) * P],
                                         rbf[:, b * P:(b + 1) * P],
                                         rbf[:, b * P:(b + 1) * P],
                                         start=True, stop=True)
                    AT = wp.tile([P, 4 * P], bf16, tag="AT")
                    nc.vector.tensor_mul(AT[:], R4[:], t1[:])
                    nc.vector.tensor_add(AT[:], AT[:], mBTs[:])

                    # degrees -> dinv -> An^T = outer(dinv) * A^T
                    dg_ps = ps.tile([1, 512], f32, tag="hh")
                    nc.tensor.matmul(dg_ps[:], onecb[:], AT[:], start=True, stop=True)
                    dr = sp.tile([1, 512], f32, tag="dr", bufs=1)
                    nc.scalar.activation(dr[:], dg_ps[:], Act.Sqrt)
                    dinv = sp.tile([1, 512], f32, tag="dinv")
                    nc.vector.reciprocal_approx_fast(dinv[:], dr[:])
                    dbf = sp.tile([1, 512], bf16, tag="ebf", bufs=2)
                    nc.vector.tensor_copy(dbf[:], dinv[:])
                    E4 = ps.tile([P, 4 * P], f32, tag="adj")
                    for b in range(4):
                        nc.tensor.matmul(E4[:, b * P:(b + 1) * P],
                                         dbf[:, b * P:(b + 1) * P],
                                         dbf[:, b * P:(b + 1) * P],
                                         start=True, stop=True)
                    nc.vector.tensor_mul(AnT[:, c0:c0 + 4 * P], E4[:], AT[:])

                    # h1 blocks: (An @ XW)^T per feature-half
                    for jh in range(JT):
                        hh4 = ps.tile([128, 4 * P], f32, tag="hh")
                        for b in range(4):
                            cb = c0 + b * P
                            blk = sb * 4 + b
                            nc.tensor.matmul(
                                hh4[:, b * P:(b + 1) * P],
                                xw1[:, blk * H + jh * 128:blk * H + (jh + 1) * 128],
                                AnT[:, cb:cb + P],
                                start=True, stop=True,
                            )
                        hh_copy(h1T, jh, c0, hh4, st1p, sb)

                bn_gcn(h1T, st1p, st1_in, st1_out, g1s, be1s)

                # ======== layer 2 ========
                xw2 = xw_layer(h1T, W2s, b2s, "xw2")
                st2p = sp.tile([128, 32], f32, tag="stp", bufs=1)
                dma_engs = [nc.sync, nc.scalar, nc.gpsimd]
                for sb in range(SB):
                    c0 = sb * 4 * P
                    for jh in range(JT):
                        hh4 = ps.tile([128, 4 * P], f32, tag="hh")
                        for b in range(4):
                            cb = c0 + b * P
                            blk = sb * 4 + b
                            nc.tensor.matmul(
                                hh4[:, b * P:(b + 1) * P],
                                xw2[:, blk * H + jh * 128:blk * H + (jh + 1) * 128],
                                AnT[:, cb:cb + P],
                                start=True, stop=True,
                            )
                        hh_copy(h2T, jh, c0, hh4, st2p, sb)
                        # export this sample's pre-BN h2 slice (BN applied on
                        # the import side: per-feature affine commutes with
                        # the redistribution)
                        eng = dma_engs[(sb * 2 + jh) % 2]
                        eng.dma_start(
                            out=a2a_in[jh][:, sb].rearrange("cd f t -> f cd t"),
                            in_=h2T[jh][:, c0:c0 + 512].rearrange(
                                "f (cd t) -> f cd t", t=TSL))

                # stats AR first on the CC queue so BN params are ready when
                # the imports land; then the two AllToAll halves
                stg2 = bn_stats_ar(st2p, st2_in, st2_out)
                for jh in range(JT):
                    cc("AllToAll", Alu.bypass, a2a_in[jh], a2a_out[jh])

                z1p = ps.tile([128, B], f32, tag="G")
                g2bf = []
                for jh in range(JT):
                    a2, c2 = bn_params(stg2, g2s, be2s, jh)
                    gt = pp.tile([128, T], bf16, tag=f"xT{jh}", name=f"g2bf{jh}")
                    gtv = gt.rearrange("f (r s t) -> f r s t", r=NCORES, s=BLOC)
                    a2v = a2a_out[jh].rearrange("r s f t -> f r s t")
                    nc.sync.dma_start(out=gtv[:, :4], in_=a2v[:, :4])
                    nc.scalar.dma_start(out=gtv[:, 4:], in_=a2v[:, 4:])
                    nc.scalar.activation(gt[:], gt[:], Act.Relu,
                                         bias=c2[:], scale=a2[:])
                    g2bf.append(gt)
                    for tl in range(TSL):
                        idx = tl * JT + jh
                        rhs = gt.rearrange("f (b t) -> f t b", t=TSL)[:, tl, :]
                        nc.tensor.matmul(
                            z1p[:], wm1s[:, idx * 128:(idx + 1) * 128], rhs,
                            start=(jh == 0 and tl == 0),
                            stop=(jh == JT - 1 and tl == TSL - 1))
                z1s = sp.tile([128, B], f32, tag="z1s")
                nc.vector.tensor_copy(z1s[:], z1p[:])
                nc.gpsimd.dma_start(out=z1_in[:], in_=z1s[:])
                cc("AllReduce", Alu.add, z1_in, z1_out)
                z1g = sp.tile([128, B], f32, tag="z1g")
                nc.gpsimd.dma_start(out=z1g[:], in_=z1_out[:])

                # ======== head BN + relu (replicated) ========
                def head_bn(zt, parts, gs, bes):
                    stm = sp.tile([parts, 1], f32, tag="hstm")
                    nc.vector.reduce_sum(stm[:], zt[:], AX.X)
                    mean = sp.tile([parts, 1], f32, tag="hmean")
                    nc.vector.tensor_scalar_mul(mean[:], stm[:], 1.0 / CNT2)
                    sqs2 = sp.tile([parts, 64], f32, tag="hsq")
                    sts = sp.tile([parts, 1], f32, tag="hsts")
                    nc.scalar.activation(sqs2[:], zt[:], Act.Square, accum_out=sts[:])
                    var = sp.tile([parts, 1], f32, tag="hvar")
                    nc.vector.tensor_scalar_mul(var[:], sts[:], 1.0 / CNT2)
                    msq = sp.tile([parts, 1], f32, tag="hmsq")
                    nc.vector.tensor_mul(msq[:], mean[:], mean[:])
                    nc.vector.tensor_sub(var[:], var[:], msq[:])
                    sd = sp.tile([parts, 1], f32, tag="hsd")
                    nc.scalar.activation(sd[:], var[:], Act.Sqrt,
                                         bias=epsb[:var.shape[0], :])
                    rsd = sp.tile([parts, 1], f32, tag="hrsd")
                    nc.vector.reciprocal(rsd[:], sd[:])
                    a = sp.tile([parts, 1], f32, tag="ha")
                    nc.vector.tensor_mul(a[:], gs[:], rsd[:])
                    c = sp.tile([parts, 1], f32, tag="hc")
                    nc.vector.tensor_mul(c[:], mean[:], a[:])
                    nc.vector.tensor_sub(c[:], bes[:], c[:])
                    nc.scalar.activation(zt[:], zt[:], Act.Relu, bias=c[:], scale=a[:])

                head_bn(z1g, 128, gm1s, bem1s)

                z2_ps = ps.tile([64, B], f32, tag="adj")
                nc.tensor.matmul(z2_ps[:], Wm2s[:], z1g[:], start=True, stop=True)
                z2t = sp.tile([64, B], f32, tag="z2t")
                nc.vector.tensor_copy(z2t[:], z2_ps[:])
                head_bn(z2t, 64, gm2s, bem2s)

                z3_ps = ps.tile([2, B], f32, tag="adj")
                nc.tensor.matmul(z3_ps[:], Wm3s[:], z2t[:], start=True, stop=True)
                z3 = sp.tile([2, B], f32, tag="z3")
                nc.vector.tensor_scalar_add(z3[:], z3_ps[:], bm3s[:])
                nc.gpsimd.dma_start(out=out_ext[:], in_=z3[:])

    nc.finalize()
    return nc


_CACHE = {}


def prepare_in_maps(inputs):
    import ml_dtypes
    bf16 = ml_dtypes.bfloat16

    x = np.asarray(inputs["x"], np.float32)
    mask = np.asarray(inputs["edge_prior_mask"], np.float32)
    Wm1 = np.asarray(inputs["Wm1"], np.float32)

    mA = 0.5 * mask * (1.0 - np.eye(P, dtype=np.float32))
    mB = mA + np.eye(P, dtype=np.float32)

    def c2(v, parts):  # [2*parts] -> [parts, 2] column-per-tile packing
        return np.ascontiguousarray(
            np.asarray(v, np.float32).reshape(2, parts).T)

    common = {
        "W1": np.asarray(inputs["W1"], np.float32).astype(bf16),
        "b1r": np.asarray(inputs["b1"], np.float32).reshape(1, H).astype(bf16),
        "g1p": c2(inputs["g1"], 128), "be1p": c2(inputs["be1"], 128),
        "W2": np.asarray(inputs["W2"], np.float32).astype(bf16),
        "b2r": np.asarray(inputs["b2"], np.float32).reshape(1, H).astype(bf16),
        "g2p": c2(inputs["g2"], 128), "be2p": c2(inputs["be2"], 128),
        "mAT": np.ascontiguousarray(np.tile(mA.T, (1, 4))).astype(bf16),
        "mBT": np.ascontiguousarray(np.tile(mB.T, (1, 4))).astype(bf16),
        "gm1": np.asarray(inputs["gm1"], np.float32).reshape(128, 1),
        "bem1": np.asarray(inputs["bem1"], np.float32).reshape(128, 1),
        "Wm2": np.asarray(inputs["Wm2"], np.float32),
        "gm2": np.asarray(inputs["gm2"], np.float32).reshape(64, 1),
        "bem2": np.asarray(inputs["bem2"], np.float32).reshape(64, 1),
        "Wm3": np.asarray(inputs["Wm3"], np.float32),
        "bm3": np.asarray(inputs["bm3"], np.float32).reshape(2, 1),
        "ones_row": np.ones((1, 128), bf16),
    }
    in_maps = []
    for c in range(NCORES):
        xc = x[c * BLOC:(c + 1) * BLOC].reshape(T, D)
        m = dict(common)
        m["xT"] = np.ascontiguousarray(xc.T).astype(bf16)
        m["Wm1s"] = np.ascontiguousarray(
            Wm1[c * FSL:(c + 1) * FSL, :]).astype(bf16)
        in_maps.append(m)
    return in_maps


def kernel(**inputs):
    import concourse.bass_utils as bass_utils

    has_bias = bool(np.any(np.asarray(inputs["b1"])) or
                    np.any(np.asarray(inputs["b2"])))
    in_maps = prepare_in_maps(inputs)
    key = ("nc", has_bias)
    if key not in _CACHE:
        _CACHE[key] = build_bass(has_bias=has_bias)
    res = bass_utils.run_bass_kernel_spmd(
        _CACHE[key], in_maps, core_ids=list(range(NCORES)))
    _CACHE["last"] = res
    out = res.results[0]["out"]  # [2, B]
    return np.ascontiguousarray(np.asarray(out).T)
